# revision 30
# baseline (speedup 1.0000x reference)
"""Bass/Trainium2 kernel for nn_DotProductAttention_47528108097846.

reference:
    scores = einsum('bhqd,bhkd->bhqk', Q, K) / 16
    attn = softmax(scores, axis=-1)
    h = einsum('bhqk,bhkd->bhqd', attn, V)
    return reshape(h, (S, B, H, D))

B=2, H=8, S=4096, D=64. 16 (b,h) pairs sharded as 2 per NeuronCore across 8
cores (batch+head parallel, no cross-core comms).

Per-core design (v4 — PE row tiling, dual-engine exp, pair-tile scores):
  - All main-loop matmuls use 64x128 PE row tiles (contraction=64), so the
    two half-arrays T0 (SBUF partitions 0-63) and T8 (64-127) run
    concurrently. KT/QT are built transposed (PE transposes through a
    borrowed scores-pool PSUM slot) in partitions 0-63 and duplicated into
    64-127 by an SBUF->SBUF DMA.
  - QK: kb-pair per step — T0 computes block 2p's scoresT [128, 512] into
    bank 0 of a [128, 2, 512] pair tile while T8 computes block 2p+1 into
    bank 1. K is pre-scaled by 1/256 at cast time so scores arrive as
    s/256. Pair tiles are triple-buffered (12KB PSUM), which hides the
    QK->exp->reuse round trip.
  - exp: one N=1024 instruction per pair, split across two engines:
    ScalarE ACTIVATE Exp(scale=16) takes 9/16 of pairs; 7/16 run on the
    Vector engine via a custom DVE op EXP_POW16_ANT:
    (1 + u + u^2/2)^16 = exp(16u)*(1 - (16u)^3/1536 ...) — exactly 8 ALU
    stages; rel err ~2e-4 typical, ~1e-2 at 6-sigma scores (negligible
    after softmax).
  - AV non-transposed: out[q, c] += eT[k, q]^T @ V'[k, c] with eT slices
    as stationary weights and V' (64 V columns + ones column for the
    softmax denominator) streamed 65 wide. T0 takes keys 0-63, T8 keys
    64-127, accumulating into separate 1-bank PSUM tiles merged in the
    epilogue. All 128 output partitions used; no output transpose.
    start=True resets a whole 2KB PSUM bank, so only the first matmul
    touching each bank carries it.
  - Software pipelining: AV(p-2) is emitted after QK(p) so the in-order PE
    queue never stalls on the exp engines; eT pool is 8 deep so exp never
    waits on AV either. Prologue pieces (load+cast+transpose+dup per
    CH=8-block group) are scheduled into specific (head, qgroup, pair)
    slots to overlap the whole K/Q/V preparation with compute.
  - Epilogue per 512-query group: ScalarE evacuates accumulator A,
    VectorE adds B, one reciprocal + one broadcast tensor_mul normalizes,
    DMA to DRAM.

PSUM budget/partition: scores 3 bufs x 4KB + AV accumulators 2 x 2KB = 16KB.
Measured on trn2: ~250 us/core device time (baseline 334 us), l2 rel err
2.7e-3. Engines in steady state: ScalarE ~85%, VectorE ~80%, PE ~72%.
"""
import numpy as np

import concourse.bass as bass
import concourse.bacc as bacc
import concourse.tile as tile
from concourse import mybir
from concourse.masks import make_identity
from concourse.bass_utils import run_bass_kernel_spmd

B, H, S, D = 2, 8, 4096, 64
N_CORES = 8
PAIRS_PER_CORE = (B * H) // N_CORES  # 2 heads per core

f32 = mybir.dt.float32
bf16 = mybir.dt.bfloat16

QG = 512             # queries per score tile
NQG = S // QG        # 8 q-groups per head
NKB = S // 128       # 32 key-blocks per head
NPR = NKB // 2       # 16 key-block pairs per q-group pass
NQB = QG // 128      # 4 query-blocks per q-group

# kb-pairs (mod 16) whose exp runs on the Vector engine (custom DVE op); the
# rest use ScalarE ACTIVATE. 7/16 keeps the two engines' total load balanced.
DVE_PRS = frozenset((1, 3, 5, 8, 10, 12, 14))


# --------------- custom DVE exp op (registered once at import) -----------
def _register_exp_op():
    import concourse.dve_ops as dve_ops
    for op in dve_ops.OPS:
        if op.name == "EXP_POW16_ANT":
            return op
    from concourse.dve_spec import Spec, Src0, C1, One, sq, lower
    from concourse.dve_uop import DveOpSpec

    u = Src0
    body = sq(sq(sq(sq((One + u) + sq(u) * C1))))

    def _ref(in0, in1, s0, s1, imm2):
        uu = np.asarray(in0, dtype=np.float32)
        p = ((1.0 + uu) + uu * uu * np.float32(s1)).astype(np.float32)
        for _ in range(4):
            p = (p * p).astype(np.float32)
        return p

    spec = Spec(body=body, reference=_ref)
    opcode = dve_ops._CUSTOM_DVE_ROW_BASE + len(dve_ops.OPS)
    shas = {}
    for ver in ("v3", "v4"):
        tmp = DveOpSpec(name="EXP_POW16_ANT", opcode=opcode,
                        uops=lower(spec, ver=ver), rd1_en=False)
        shas[ver] = tmp.sha(ver)
    op = dve_ops.DveOp("EXP_POW16_ANT", spec, subdim=False, uops_sha=shas)
    dve_ops.OPS.append(op)
    dve_ops.CUSTOM_DVE_SPECS["EXP_POW16_ANT"] = spec
    dve_ops._SUB_OPCODE_FOR_NAME["EXP_POW16_ANT"] = opcode
    return op


EXP_OP = _register_exp_op()


def build_attention(nc, tc, q, k, v, o):
    import contextlib
    ctx = contextlib.ExitStack()
    consts = ctx.enter_context(tc.tile_pool(name="consts", bufs=1))
    nat = ctx.enter_context(tc.tile_pool(name="nat", bufs=3))
    persist = ctx.enter_context(tc.tile_pool(name="persist", bufs=1))
    sb = ctx.enter_context(tc.tile_pool(name="sb", bufs=4))
    pool_e = ctx.enter_context(tc.tile_pool(name="sb_e", bufs=8))
    pool_s = ctx.enter_context(tc.tile_pool(name="ps_s", bufs=3, space="PSUM"))
    pool_o = ctx.enter_context(tc.tile_pool(name="ps_o", bufs=1, space="PSUM"))

    ident = consts.tile([128, 128], f32)
    make_identity(nc, ident)
    identb = consts.tile([128, 128], bf16)
    nc.vector.tensor_copy(out=identb, in_=ident)

    qts, kts, v1s = [], [], []
    CH = 8

    def alloc_head(h):
        qt = persist.tile([128, NKB, 128], bf16, tag=f"qt{h}")
        kt = persist.tile([128, NKB, 128], bf16, tag=f"kt{h}")
        v1 = persist.tile([128, NKB, 65], bf16, tag=f"v1{h}")
        qts.append(qt)
        kts.append(kt)
        v1s.append(v1)

    def emit_kgroup(h, g):
        """K chunk-group via PE transposes (borrows one scores-pool PSUM
        slot) + partition-dup DMA so both PE half-arrays see every kb."""
        kt = kts[h]
        natc = nat.tile([128, CH, 64], f32, tag="natx")
        nc.sync.dma_start(
            out=natc,
            in_=k[h].rearrange("(n p) d -> p n d", p=128)[
                :, g * CH:(g + 1) * CH, :])
        natbc = nat.tile([128, CH, 64], bf16, tag="natbx")
        # K pre-scaled by 1/256 so scores arrive as s/256 (power-of-two:
        # exact in bf16); DVE — gpsimd's scaled-cast path is ~10x slower
        nc.vector.tensor_scalar_mul(out=natbc, in0=natc, scalar1=1.0 / 256.0)
        ps_tr = pool_s.tile([64, CH, 128], bf16, tag="s")
        for j in range(CH):
            nc.tensor.transpose(ps_tr[:, j, :], natbc[:, j, :], identb)
        nc.vector.tensor_copy(
            out=kt[0:64, g * CH:(g + 1) * CH, :], in_=ps_tr)
        nc.sync.dma_start(out=kt[64:128, g * CH:(g + 1) * CH, :],
                          in_=kt[0:64, g * CH:(g + 1) * CH, :])

    def emit_qgroup(h, g, cast_dve=False):
        """Q chunk-group via PE transposes (borrows a scores-pool PSUM
        slot — place at qgroup boundaries) + partition-dup DMA for T8."""
        qt = qts[h]
        natc = nat.tile([128, CH, 64], f32, tag="nat")
        nc.sync.dma_start(
            out=natc,
            in_=q[h].rearrange("(n p) d -> p n d", p=128)[
                :, g * CH:(g + 1) * CH, :])
        natbc = nat.tile([128, CH, 64], bf16, tag="natb")
        if cast_dve:
            nc.vector.tensor_copy(out=natbc, in_=natc)
        else:
            nc.gpsimd.tensor_copy(out=natbc, in_=natc)
        ps_tr = pool_s.tile([64, CH, 128], bf16, tag="s")
        for j in range(CH):
            nc.tensor.transpose(ps_tr[:, j, :], natbc[:, j, :], identb)
        nc.vector.tensor_copy(
            out=qt[0:64, g * CH:(g + 1) * CH, :], in_=ps_tr)
        nc.sync.dma_start(out=qt[64:128, g * CH:(g + 1) * CH, :],
                          in_=qt[0:64, g * CH:(g + 1) * CH, :])

    def emit_v(h):
        v1 = v1s[h]
        vnat = nat.tile([128, NKB, 64], f32, tag="vnat")
        nc.sync.dma_start(
            out=vnat, in_=v[h].rearrange("(n p) d -> p n d", p=128))
        nc.gpsimd.memset(v1[:, :, 64:65], 1.0)
        nc.gpsimd.tensor_copy(out=v1[:, :, 0:64], in_=vnat)

    # Fast start: head 0's Q group 0 via PE transposes + all its K groups
    # via background crossbar DMAs; everything else streams in behind the
    # main loop, a few kb-slots before first use.
    alloc_head(0)
    alloc_head(1)
    emit_kgroup(0, 0)
    emit_qgroup(0, 0, cast_dve=True)
    emit_v(0)
    emit_kgroup(0, 1)

    PIECES = {
        (0, 0, 4): [lambda: emit_kgroup(0, 2)],
        (0, 0, 8): [lambda: emit_kgroup(0, 3)],
        (0, 0, 12): [lambda: emit_qgroup(0, 1)],
        (0, 2, 4): [lambda: emit_qgroup(0, 2)],
        (0, 3, 8): [lambda: emit_kgroup(1, 0)],
        (0, 4, 4): [lambda: emit_qgroup(0, 3)],
        (0, 4, 12): [lambda: emit_kgroup(1, 1)],
        (0, 5, 8): [lambda: emit_kgroup(1, 2), lambda: emit_v(1)],
        (0, 6, 8): [lambda: emit_qgroup(1, 0)],
        (0, 6, 12): [lambda: emit_kgroup(1, 3)],
        (0, 7, 8): [lambda: emit_qgroup(1, 1)],
        (1, 1, 8): [lambda: emit_qgroup(1, 2)],
        (1, 3, 8): [lambda: emit_qgroup(1, 3)],
    }

    def main_compute():
        for h in range(PAIRS_PER_CORE):
            qt, kt, v1 = qts[h], kts[h], v1s[h]
            qtf = qt.rearrange("p n d -> p (n d)")
            out_r = o[h].rearrange("(n p) d -> p n d", p=128)
            for qg in range(NQG):
                ps_oa = pool_o.tile([128, NQB, 128], f32, tag="oa")
                ps_ob = pool_o.tile([128, NQB, 128], f32, tag="ob")

                def emit_av(eT, pr):
                    for kk in range(2):
                        kb = 2 * pr + kk
                        last = (kb == NKB - 1)
                        for qb in range(NQB):
                            # start=True resets the whole 2KB PSUM bank (4
                            # qb slots), so only the first matmul touching
                            # each bank may carry it; the rest accumulate
                            # onto the bank-wide zeros.
                            first = (kb < 2) and (qb == 0)
                            nc.tensor.matmul(
                                out=ps_oa[:, qb, 0:65],
                                lhsT=eT[0:64,
                                        kk * QG + qb * 128:
                                        kk * QG + (qb + 1) * 128],
                                rhs=v1[0:64, kb, :],
                                start=first and kk == 0, stop=last,
                                tile_position=(0, 0), skip_group_check=True)
                            nc.tensor.matmul(
                                out=ps_ob[:, qb, 0:65],
                                lhsT=eT[64:128,
                                        kk * QG + qb * 128:
                                        kk * QG + (qb + 1) * 128],
                                rhs=v1[64:128, kb, :],
                                start=first and kk == 0, stop=last,
                                tile_position=(64, 0), skip_group_check=True)

                # software pipeline depth 2 pairs: AV(pr-2) is emitted
                # after QK(pr), so by the time the in-order PE queue
                # reaches an AV group its exp has long finished.
                pend = []
                for pr in range(NPR):
                    for fn in PIECES.get((h, qg, pr), ()):
                        fn()
                    # pair tile: bank 0 = even kb scores, bank 1 = odd kb;
                    # T0/T8 fill the two banks concurrently, one exp
                    # instruction covers the pair at full N=1024 width.
                    ps_s = pool_s.tile([128, 2, QG], f32, tag="s")
                    nc.tensor.matmul(
                        out=ps_s[:, 0, :],
                        lhsT=kt[0:64, 2 * pr, :],
                        rhs=qtf[0:64, qg * QG:(qg + 1) * QG],
                        start=True, stop=True, tile_position=(0, 0))
                    nc.tensor.matmul(
                        out=ps_s[:, 1, :],
                        lhsT=kt[64:128, 2 * pr + 1, :],
                        rhs=qtf[64:128, qg * QG:(qg + 1) * QG],
                        start=True, stop=True, tile_position=(64, 0))
                    eT = pool_e.tile([128, 2 * QG], bf16, tag="exp")
                    psf = ps_s.rearrange("p a b -> p (a b)")
                    if (pr % 16) in DVE_PRS:
                        nc.vector._custom_dve(EXP_OP, out=eT, in0=psf, s1=0.5)
                    else:
                        nc.scalar.activation(
                            out=eT, in_=psf,
                            func=mybir.ActivationFunctionType.Exp,
                            scale=16.0)
                    pend.append((eT, pr))
                    if len(pend) > 4:
                        emit_av(*pend.pop(0))
                for p in pend:
                    emit_av(*p)

                # epilogue: merge T0+T8 accumulators, normalize, store
                om = sb.tile([128, NQB, 65], f32, tag="om")
                nc.scalar.copy(out=om, in_=ps_oa[:, :, 0:65])
                nc.vector.tensor_add(
                    out=om, in0=om, in1=ps_ob[:, :, 0:65])
                osb = sb.tile([128, NQB, 64], f32, tag="osb")
                rcp = sb.tile([128, NQB], f32, tag="rcp")
                nc.vector.reciprocal(out=rcp, in_=om[:, :, 64])
                xb, rb = bass.broadcast_tensor_aps(
                    om[:, :, 0:64],
                    rcp.rearrange("p (a b) -> p a b", b=1))
                nc.vector.tensor_mul(out=osb, in0=xb, in1=rb)
                nc.sync.dma_start(
                    out=out_r[:, qg * NQB:(qg + 1) * NQB, :],
                    in_=osb)


    main_compute()
    ctx.close()


_CACHED = {}


def build_program():
    key = "v2"
    if key in _CACHED:
        return _CACHED[key]
    nc = bacc.Bacc("TRN2", target_bir_lowering=False, debug=False,
                   num_devices=N_CORES)
    q = nc.dram_tensor("q", [PAIRS_PER_CORE, S, D], f32,
                       kind="ExternalInput").ap()
    k = nc.dram_tensor("k", [PAIRS_PER_CORE, S, D], f32,
                       kind="ExternalInput").ap()
    v = nc.dram_tensor("v", [PAIRS_PER_CORE, S, D], f32,
                       kind="ExternalInput").ap()
    o = nc.dram_tensor("o", [PAIRS_PER_CORE, S, D], f32,
                       kind="ExternalOutput").ap()
    with tile.TileContext(nc) as tc:
        build_attention(nc, tc, q, k, v, o)
    nc.compile()
    _CACHED[key] = nc
    return nc


def kernel(queries, keys, values, adj=None, **_unused):
    """Full-input attention on 8 NeuronCores. Returns [S, B, H, D] fp32."""
    queries = np.ascontiguousarray(queries, dtype=np.float32)
    keys = np.ascontiguousarray(keys, dtype=np.float32)
    values = np.ascontiguousarray(values, dtype=np.float32)

    nc = build_program()
    qf = queries.reshape(B * H, S, D)
    kf = keys.reshape(B * H, S, D)
    vf = values.reshape(B * H, S, D)
    in_maps = []
    for c in range(N_CORES):
        sl = slice(c * PAIRS_PER_CORE, (c + 1) * PAIRS_PER_CORE)
        in_maps.append({"q": qf[sl], "k": kf[sl], "v": vf[sl]})
    res = run_bass_kernel_spmd(nc, in_maps, list(range(N_CORES)))
    hout = np.empty((B * H, S, D), dtype=np.float32)
    for c in range(N_CORES):
        hout[c * PAIRS_PER_CORE:(c + 1) * PAIRS_PER_CORE] = res.results[c]["o"]
    return hout.reshape(B, H, S, D).reshape(S, B, H, D)


# revision 32
# speedup vs baseline: 1.0042x; 1.0042x over previous
"""Bass/Trainium2 kernel for nn_DotProductAttention_47528108097846.

reference:
    scores = einsum('bhqd,bhkd->bhqk', Q, K) / 16
    attn = softmax(scores, axis=-1)
    h = einsum('bhqk,bhkd->bhqd', attn, V)
    return reshape(h, (S, B, H, D))

B=2, H=8, S=4096, D=64. 16 (b,h) pairs sharded as 2 per NeuronCore across 8
cores (batch+head parallel, no cross-core comms).

Per-core design (v4 — PE row tiling, dual-engine exp, pair-tile scores):
  - All main-loop matmuls use 64x128 PE row tiles (contraction=64), so the
    two half-arrays T0 (SBUF partitions 0-63) and T8 (64-127) run
    concurrently. KT/QT are built transposed (PE transposes through a
    borrowed scores-pool PSUM slot) in partitions 0-63 and duplicated into
    64-127 by an SBUF->SBUF DMA.
  - QK: kb-pair per step — T0 computes block 2p's scoresT [128, 512] into
    bank 0 of a [128, 2, 512] pair tile while T8 computes block 2p+1 into
    bank 1. K is pre-scaled by 1/256 at cast time so scores arrive as
    s/256. Pair tiles are triple-buffered (12KB PSUM), which hides the
    QK->exp->reuse round trip.
  - exp: one N=1024 instruction per pair, split across two engines:
    ScalarE ACTIVATE Exp(scale=16) takes 9/16 of pairs; 7/16 run on the
    Vector engine via a custom DVE op EXP_POW16_ANT:
    (1 + u + u^2/2)^16 = exp(16u)*(1 - (16u)^3/1536 ...) — exactly 8 ALU
    stages; rel err ~2e-4 typical, ~1e-2 at 6-sigma scores (negligible
    after softmax).
  - AV non-transposed: out[q, c] += eT[k, q]^T @ V'[k, c] with eT slices
    as stationary weights and V' (64 V columns + ones column for the
    softmax denominator) streamed 65 wide. T0 takes keys 0-63, T8 keys
    64-127, accumulating into separate 1-bank PSUM tiles merged in the
    epilogue. All 128 output partitions used; no output transpose.
    start=True resets a whole 2KB PSUM bank, so only the first matmul
    touching each bank carries it.
  - Software pipelining: AV(p-2) is emitted after QK(p) so the in-order PE
    queue never stalls on the exp engines; eT pool is 8 deep so exp never
    waits on AV either. Prologue pieces (load+cast+transpose+dup per
    CH=8-block group) are scheduled into specific (head, qgroup, pair)
    slots to overlap the whole K/Q/V preparation with compute.
  - Epilogue per 512-query group: ScalarE evacuates accumulator A,
    VectorE adds B, one reciprocal + one broadcast tensor_mul normalizes,
    DMA to DRAM.

PSUM budget/partition: scores 3 bufs x 4KB + AV accumulators 2 x 2KB = 16KB.
Measured on trn2: ~250 us/core device time (baseline 334 us), l2 rel err
2.7e-3. Engines in steady state: ScalarE ~85%, VectorE ~80%, PE ~72%.
"""
import numpy as np

import concourse.bass as bass
import concourse.bacc as bacc
import concourse.tile as tile
from concourse import mybir
from concourse.masks import make_identity
from concourse.bass_utils import run_bass_kernel_spmd

B, H, S, D = 2, 8, 4096, 64
N_CORES = 8
PAIRS_PER_CORE = (B * H) // N_CORES  # 2 heads per core

f32 = mybir.dt.float32
bf16 = mybir.dt.bfloat16

QG = 512             # queries per score tile
NQG = S // QG        # 8 q-groups per head
NKB = S // 128       # 32 key-blocks per head
NPR = NKB // 2       # 16 key-block pairs per q-group pass
NQB = QG // 128      # 4 query-blocks per q-group

# kb-pairs (mod 16) whose exp runs on the Vector engine (custom DVE op); the
# rest use ScalarE ACTIVATE. 7/16 keeps the two engines' total load balanced.
DVE_PRS = frozenset((1, 3, 5, 8, 10, 12, 14))


# --------------- custom DVE exp op (registered once at import) -----------
def _register_exp_op():
    import concourse.dve_ops as dve_ops
    for op in dve_ops.OPS:
        if op.name == "EXP_POW16_ANT":
            return op
    from concourse.dve_spec import Spec, Src0, C1, One, sq, lower
    from concourse.dve_uop import DveOpSpec

    u = Src0
    body = sq(sq(sq(sq((One + u) + sq(u) * C1))))

    def _ref(in0, in1, s0, s1, imm2):
        uu = np.asarray(in0, dtype=np.float32)
        p = ((1.0 + uu) + uu * uu * np.float32(s1)).astype(np.float32)
        for _ in range(4):
            p = (p * p).astype(np.float32)
        return p

    spec = Spec(body=body, reference=_ref)
    opcode = dve_ops._CUSTOM_DVE_ROW_BASE + len(dve_ops.OPS)
    shas = {}
    for ver in ("v3", "v4"):
        tmp = DveOpSpec(name="EXP_POW16_ANT", opcode=opcode,
                        uops=lower(spec, ver=ver), rd1_en=False)
        shas[ver] = tmp.sha(ver)
    op = dve_ops.DveOp("EXP_POW16_ANT", spec, subdim=False, uops_sha=shas)
    dve_ops.OPS.append(op)
    dve_ops.CUSTOM_DVE_SPECS["EXP_POW16_ANT"] = spec
    dve_ops._SUB_OPCODE_FOR_NAME["EXP_POW16_ANT"] = opcode
    return op


EXP_OP = _register_exp_op()


def build_attention(nc, tc, q, k, v, o):
    import contextlib
    ctx = contextlib.ExitStack()
    consts = ctx.enter_context(tc.tile_pool(name="consts", bufs=1))
    nat = ctx.enter_context(tc.tile_pool(name="nat", bufs=3))
    persist = ctx.enter_context(tc.tile_pool(name="persist", bufs=1))
    sb = ctx.enter_context(tc.tile_pool(name="sb", bufs=4))
    pool_e = ctx.enter_context(tc.tile_pool(name="sb_e", bufs=8))
    pool_s = ctx.enter_context(tc.tile_pool(name="ps_s", bufs=3, space="PSUM"))
    pool_o = ctx.enter_context(tc.tile_pool(name="ps_o", bufs=1, space="PSUM"))

    ident = consts.tile([128, 128], f32)
    make_identity(nc, ident)
    identb = consts.tile([128, 128], bf16)
    nc.vector.tensor_copy(out=identb, in_=ident)

    qts, kts, v1s = [], [], []
    CH = 8

    def alloc_head(h):
        qt = persist.tile([128, NKB, 128], bf16, tag=f"qt{h}")
        kt = persist.tile([128, NKB, 128], bf16, tag=f"kt{h}")
        v1 = persist.tile([128, NKB, 65], bf16, tag=f"v1{h}")
        qts.append(qt)
        kts.append(kt)
        v1s.append(v1)

    def emit_kgroup(h, g):
        """K chunk-group via PE transposes (borrows one scores-pool PSUM
        slot) + partition-dup DMA so both PE half-arrays see every kb."""
        kt = kts[h]
        natc = nat.tile([128, CH, 64], f32, tag="natx")
        nc.sync.dma_start(
            out=natc,
            in_=k[h].rearrange("(n p) d -> p n d", p=128)[
                :, g * CH:(g + 1) * CH, :])
        natbc = nat.tile([128, CH, 64], bf16, tag="natbx")
        # K pre-scaled by 1/256 so scores arrive as s/256 (power-of-two:
        # exact in bf16); DVE — gpsimd's scaled-cast path is ~10x slower
        nc.vector.tensor_scalar_mul(out=natbc, in0=natc, scalar1=1.0 / 256.0)
        ps_tr = pool_s.tile([64, CH, 128], bf16, tag="s")
        for j in range(CH):
            nc.tensor.transpose(ps_tr[:, j, :], natbc[:, j, :], identb)
        nc.vector.tensor_copy(
            out=kt[0:64, g * CH:(g + 1) * CH, :], in_=ps_tr)
        nc.sync.dma_start(out=kt[64:128, g * CH:(g + 1) * CH, :],
                          in_=kt[0:64, g * CH:(g + 1) * CH, :])

    def emit_qgroup(h, g, cast_dve=False):
        """Q chunk-group via PE transposes (borrows a scores-pool PSUM
        slot — place at qgroup boundaries) + partition-dup DMA for T8."""
        qt = qts[h]
        natc = nat.tile([128, CH, 64], f32, tag="nat")
        nc.sync.dma_start(
            out=natc,
            in_=q[h].rearrange("(n p) d -> p n d", p=128)[
                :, g * CH:(g + 1) * CH, :])
        natbc = nat.tile([128, CH, 64], bf16, tag="natb")
        if cast_dve:
            nc.vector.tensor_copy(out=natbc, in_=natc)
        else:
            nc.gpsimd.tensor_copy(out=natbc, in_=natc)
        ps_tr = pool_s.tile([64, CH, 128], bf16, tag="s")
        for j in range(CH):
            nc.tensor.transpose(ps_tr[:, j, :], natbc[:, j, :], identb)
        nc.vector.tensor_copy(
            out=qt[0:64, g * CH:(g + 1) * CH, :], in_=ps_tr)
        nc.sync.dma_start(out=qt[64:128, g * CH:(g + 1) * CH, :],
                          in_=qt[0:64, g * CH:(g + 1) * CH, :])

    def emit_v(h):
        v1 = v1s[h]
        vnat = nat.tile([128, NKB, 64], f32, tag="vnat")
        nc.sync.dma_start(
            out=vnat, in_=v[h].rearrange("(n p) d -> p n d", p=128))
        nc.gpsimd.memset(v1[:, :, 64:65], 1.0)
        nc.gpsimd.tensor_copy(out=v1[:, :, 0:64], in_=vnat)

    # Fast start: head 0's Q group 0 via PE transposes + all its K groups
    # via background crossbar DMAs; everything else streams in behind the
    # main loop, a few kb-slots before first use.
    alloc_head(0)
    alloc_head(1)
    emit_kgroup(0, 0)
    emit_qgroup(0, 0, cast_dve=True)
    emit_v(0)
    emit_kgroup(0, 1)

    PIECES = {
        (0, 0, 4): [lambda: emit_kgroup(0, 2)],
        (0, 0, 8): [lambda: emit_kgroup(0, 3)],
        (0, 0, 12): [lambda: emit_qgroup(0, 1)],
        (0, 2, 4): [lambda: emit_qgroup(0, 2)],
        (0, 3, 8): [lambda: emit_kgroup(1, 0)],
        (0, 4, 4): [lambda: emit_qgroup(0, 3)],
        (0, 4, 12): [lambda: emit_kgroup(1, 1)],
        (0, 5, 8): [lambda: emit_kgroup(1, 2), lambda: emit_v(1)],
        (0, 6, 8): [lambda: emit_qgroup(1, 0)],
        (0, 6, 12): [lambda: emit_kgroup(1, 3)],
        (0, 7, 8): [lambda: emit_qgroup(1, 1)],
        (1, 1, 8): [lambda: emit_qgroup(1, 2)],
        (1, 3, 8): [lambda: emit_qgroup(1, 3)],
    }

    def main_compute():
        for h in range(PAIRS_PER_CORE):
            qt, kt, v1 = qts[h], kts[h], v1s[h]
            qtf = qt.rearrange("p n d -> p (n d)")
            out_r = o[h].rearrange("(n p) d -> p n d", p=128)
            for qg in range(NQG):
                ps_oa = pool_o.tile([128, NQB, 128], f32, tag="oa")
                ps_ob = pool_o.tile([128, NQB, 128], f32, tag="ob")

                def emit_av(eT, pr):
                    for kk in range(2):
                        kb = 2 * pr + kk
                        last = (kb == NKB - 1)
                        for qb in range(NQB):
                            # start=True resets the whole 2KB PSUM bank (4
                            # qb slots), so only the first matmul touching
                            # each bank may carry it; the rest accumulate
                            # onto the bank-wide zeros.
                            first = (kb < 2) and (qb == 0)
                            nc.tensor.matmul(
                                out=ps_oa[:, qb, 0:65],
                                lhsT=eT[0:64,
                                        kk * QG + qb * 128:
                                        kk * QG + (qb + 1) * 128],
                                rhs=v1[0:64, kb, :],
                                start=first and kk == 0, stop=last,
                                tile_position=(0, 0), skip_group_check=True)
                            nc.tensor.matmul(
                                out=ps_ob[:, qb, 0:65],
                                lhsT=eT[64:128,
                                        kk * QG + qb * 128:
                                        kk * QG + (qb + 1) * 128],
                                rhs=v1[64:128, kb, :],
                                start=first and kk == 0, stop=last,
                                tile_position=(64, 0), skip_group_check=True)

                # software pipeline depth 2 pairs: AV(pr-2) is emitted
                # after QK(pr), so by the time the in-order PE queue
                # reaches an AV group its exp has long finished.
                pend = []
                for pr in range(NPR):
                    for fn in PIECES.get((h, qg, pr), ()):
                        fn()
                    # pair tile: bank 0 = even kb scores, bank 1 = odd kb;
                    # T0/T8 fill the two banks concurrently, one exp
                    # instruction covers the pair at full N=1024 width.
                    ps_s = pool_s.tile([128, 2, QG], f32, tag="s")
                    nc.tensor.matmul(
                        out=ps_s[:, 0, :],
                        lhsT=kt[0:64, 2 * pr, :],
                        rhs=qtf[0:64, qg * QG:(qg + 1) * QG],
                        start=True, stop=True, tile_position=(0, 0))
                    nc.tensor.matmul(
                        out=ps_s[:, 1, :],
                        lhsT=kt[64:128, 2 * pr + 1, :],
                        rhs=qtf[64:128, qg * QG:(qg + 1) * QG],
                        start=True, stop=True, tile_position=(64, 0))
                    eT = pool_e.tile([128, 2 * QG], bf16, tag="exp")
                    psf = ps_s.rearrange("p a b -> p (a b)")
                    if (pr % 16) in DVE_PRS:
                        nc.vector._custom_dve(EXP_OP, out=eT, in0=psf, s1=0.5)
                    elif (pr % 16) in (7, 15):
                        # these pairs would otherwise run back-to-back on
                        # ScalarE (9 of 16 pairs can't alternate perfectly);
                        # split them across both engines to halve latency
                        nc.scalar.activation(
                            out=eT[:, 0:QG], in_=psf[:, 0:QG],
                            func=mybir.ActivationFunctionType.Exp,
                            scale=16.0)
                        nc.vector._custom_dve(
                            EXP_OP, out=eT[:, QG:2 * QG],
                            in0=psf[:, QG:2 * QG], s1=0.5)
                    else:
                        nc.scalar.activation(
                            out=eT, in_=psf,
                            func=mybir.ActivationFunctionType.Exp,
                            scale=16.0)
                    pend.append((eT, pr))
                    if len(pend) > 3:
                        emit_av(*pend.pop(0))
                for p in pend:
                    emit_av(*p)

                # epilogue: merge T0+T8 accumulators, normalize, store
                om = sb.tile([128, NQB, 65], f32, tag="om")
                nc.scalar.copy(out=om, in_=ps_oa[:, :, 0:65])
                nc.vector.tensor_add(
                    out=om, in0=om, in1=ps_ob[:, :, 0:65])
                osb = sb.tile([128, NQB, 64], f32, tag="osb")
                rcp = sb.tile([128, NQB], f32, tag="rcp")
                nc.vector.reciprocal(out=rcp, in_=om[:, :, 64])
                xb, rb = bass.broadcast_tensor_aps(
                    om[:, :, 0:64],
                    rcp.rearrange("p (a b) -> p a b", b=1))
                nc.vector.tensor_mul(out=osb, in0=xb, in1=rb)
                nc.sync.dma_start(
                    out=out_r[:, qg * NQB:(qg + 1) * NQB, :],
                    in_=osb)


    main_compute()
    ctx.close()


_CACHED = {}


def build_program():
    key = "v2"
    if key in _CACHED:
        return _CACHED[key]
    nc = bacc.Bacc("TRN2", target_bir_lowering=False, debug=False,
                   num_devices=N_CORES)
    q = nc.dram_tensor("q", [PAIRS_PER_CORE, S, D], f32,
                       kind="ExternalInput").ap()
    k = nc.dram_tensor("k", [PAIRS_PER_CORE, S, D], f32,
                       kind="ExternalInput").ap()
    v = nc.dram_tensor("v", [PAIRS_PER_CORE, S, D], f32,
                       kind="ExternalInput").ap()
    o = nc.dram_tensor("o", [PAIRS_PER_CORE, S, D], f32,
                       kind="ExternalOutput").ap()
    with tile.TileContext(nc) as tc:
        build_attention(nc, tc, q, k, v, o)
    nc.compile()
    _CACHED[key] = nc
    return nc


def kernel(queries, keys, values, adj=None, **_unused):
    """Full-input attention on 8 NeuronCores. Returns [S, B, H, D] fp32."""
    queries = np.ascontiguousarray(queries, dtype=np.float32)
    keys = np.ascontiguousarray(keys, dtype=np.float32)
    values = np.ascontiguousarray(values, dtype=np.float32)

    nc = build_program()
    qf = queries.reshape(B * H, S, D)
    kf = keys.reshape(B * H, S, D)
    vf = values.reshape(B * H, S, D)
    in_maps = []
    for c in range(N_CORES):
        sl = slice(c * PAIRS_PER_CORE, (c + 1) * PAIRS_PER_CORE)
        in_maps.append({"q": qf[sl], "k": kf[sl], "v": vf[sl]})
    res = run_bass_kernel_spmd(nc, in_maps, list(range(N_CORES)))
    hout = np.empty((B * H, S, D), dtype=np.float32)
    for c in range(N_CORES):
        hout[c * PAIRS_PER_CORE:(c + 1) * PAIRS_PER_CORE] = res.results[c]["o"]
    return hout.reshape(B, H, S, D).reshape(S, B, H, D)
